# revision 48
# baseline (speedup 1.0000x reference)
"""Trainium2 Bass kernel for PVT-style spatial-reduction multi-head attention.

Problem (hardcoded shapes, fp32 inputs):
  x [2, 4096, 512]; Wq [512,512]; Wconv [512,512,2,2] (OIHW, stride 2);
  LayerNorm over the conv's flattened spatial dim (M=1024); Wkv [1024,1024];
  attention with q [B,8,4096,64], k/v [B,8,512,64]; "faithful" reshape
  (out.transpose(0,1,3,2).reshape(B,-1,512)) before Wproj [512,512].

Sharding: 8 cores = (batch b in {0,1}) x (head-pair g in {0..3}).
Core (b,g) computes heads {2g, 2g+1} of batch b and writes output rows
[b, 1024g : 1024g+1024, :].

v2.2 structure (vs the ~113us v1; TimelineSim 165911 -> 107151 ns):
 - x is transposed + bf16-cast on the host; gamma is folded into Wkv and
   beta@Wkv+bkv precomputed on host.  24 packed DMAs (8 weights on the ACT
   HWDGE queue, 8 x-chunks + 8 outputs on SP) replace 65.
 - Softmax exp runs on ACT over merged [128,1024] 2-bank PSUM tiles; all
   other PSUM->SBUF traffic is on DVE; SBUF-only scaling on Pool (gpsimd).
 - The softmax denominator rides as a 65th PSUM partition through the AV
   matmul (ones column in V), the AV copy, and the AV transpose, so its
   extraction is free; per-chunk reciprocal + Pool scatter overlap the
   ACT-bound attention loop.
 - Q projection is computed per-chunk inside the attention loop (PE slack
   under the ACT exp bottleneck) instead of in a separate phase.
 - LayerNorm: the conv bias is skipped entirely (a per-channel constant
   along the normalized axis cancels exactly in the mean subtraction) and
   the stats/normalize read the conv PSUM tile directly; sums on DVE,
   sum-of-squares via ACT Square+accum (same ACT table as Exp), and
   rsqrt(var) computed per conv tile as sqrt(1/v) with a 2-step DVE Newton
   iteration, so each tile normalizes under the next tile's conv and ACT
   never leaves the exp table (no 1.3us table reloads on HW).
 - PSUM: pmm [128,1024]x3 bufs (matmul groups AND the z-transposes, so the
   post-conv transpose chain is not serialized by a 1-buf pool), psav
   [65,512]x1, pstp x1 (AV-transpose slots).  Two x-chunks ride the gpsimd
   SWDGE queue; tail output DMAs are split across the ACT and SP queues.
"""

import sys

sys.path.insert(0, "/opt/trn_rl_repo")

import numpy as np
import ml_dtypes

import concourse.bass as bass
import concourse.bacc as bacc
import concourse.mybir as mybir
import concourse.tile as tile
from concourse.bass_utils import run_bass_kernel_spmd

F32 = mybir.dt.float32
BF16 = mybir.dt.bfloat16
NP_BF16 = ml_dtypes.bfloat16

B, N, C = 2, 4096, 512
HH, WW, SR = 64, 64, 2
NH, HD = 8, 64
M = (HH // SR) * (WW // SR)  # 1024
UN = C
EPS = 1e-5
N_CORES = 8

# small col map
COL_BQ = 0
COL_BCONV = 1  # 4 cols
COL_BK = 5
COL_BV = 6
SMALL_COLS = 7


def _build_module(reps=1, bench_internal=False):
    nc = bacc.Bacc("TRN2", target_bir_lowering=False, debug=False)

    KIND = "Internal" if bench_internal else "ExternalInput"
    xt16 = nc.dram_tensor("xt16", [C, N], BF16, kind=KIND).ap()
    wc = nc.dram_tensor("wc", [128, 8192], BF16, kind=KIND).ap()
    wq = nc.dram_tensor("wq", [128, 512], BF16, kind=KIND).ap()
    wkv = nc.dram_tensor("wkv", [128, 2048], BF16, kind=KIND).ap()
    wproj = nc.dram_tensor("wproj", [128, 2048], BF16, kind=KIND).ap()
    small = nc.dram_tensor("small", [128, SMALL_COLS], F32, kind="ExternalInput").ap()
    eye128 = nc.dram_tensor("eye128", [128, 128], BF16, kind=KIND).ap()
    bproj16 = nc.dram_tensor("bproj16", [1, C], BF16, kind=KIND).ap()
    out = nc.dram_tensor("out", [2 * UN, C], F32, kind="ExternalOutput").ap()

    AX = mybir.AxisListType.X
    OP = mybir.AluOpType
    AF = mybir.ActivationFunctionType

    with tile.TileContext(nc) as tc:
        import contextlib

        with contextlib.ExitStack() as ctx:
            persist = ctx.enter_context(tc.tile_pool(name="persist", bufs=1))
            stage = ctx.enter_context(tc.tile_pool(name="stage", bufs=3))
            pmm = ctx.enter_context(tc.tile_pool(name="pmm", bufs=3, space="PSUM"))
            psav = ctx.enter_context(tc.tile_pool(name="psav", bufs=1, space="PSUM"))
            pstp = ctx.enter_context(tc.tile_pool(name="pstp", bufs=1, space="PSUM"))

            for _rep in range(reps):
                # ---------------- DMAs ----------------
                # conv weights split per-ot so ot0 lands first; x half-0 next.
                wc_sb = persist.tile([128, 8192], BF16, name="wc_sb", tag="wc")
                nc.scalar.dma_start(wc_sb[:, 0:2048], wc[:, 0:2048])
                xt_all = persist.tile([128, 4 * N], BF16, name="xt_all", tag="xt")
                xt_sb = [xt_all[:, k * N : (k + 1) * N] for k in range(4)]
                for k in range(4):
                    eng = nc.gpsimd if k >= 2 else nc.sync
                    eng.dma_start(
                        xt_all[:, N * k : N * k + 2048],
                        xt16[128 * k : 128 * (k + 1), 0:2048],
                    )
                for i in range(1, 4):
                    nc.scalar.dma_start(
                        wc_sb[:, 2048 * i : 2048 * (i + 1)],
                        wc[:, 2048 * i : 2048 * (i + 1)],
                    )
                for k in range(4):
                    nc.sync.dma_start(
                        xt_all[:, N * k + 2048 : N * k + 4096],
                        xt16[128 * k : 128 * (k + 1), 2048:4096],
                    )
                wq_sb = persist.tile([128, 512], BF16, name="wq_sb", tag="wq")
                nc.scalar.dma_start(wq_sb[:], wq[:, :])
                wkv_sb = persist.tile([128, 2048], BF16, name="wkv_sb", tag="wkv")
                nc.scalar.dma_start(wkv_sb[:], wkv[:, :])
                wproj_sb = persist.tile([128, 2048], BF16, name="wproj_sb", tag="wp")
                nc.scalar.dma_start(wproj_sb[:], wproj[:, :])
                small_sb = persist.tile([128, SMALL_COLS], F32, name="small_sb", tag="sm")
                nc.scalar.dma_start(small_sb[:], small[:, :])
                eye_sb = persist.tile([128, 128], BF16, name="eye_sb", tag="eye")
                nc.scalar.dma_start(eye_sb[:], eye128[:, :])
                bproj_sb = persist.tile([1, C], BF16, name="bproj_sb", tag="bpj")
                nc.scalar.dma_start(bproj_sb[:], bproj16[:, :])
                ones1_sb = persist.tile([1, 128], BF16, name="ones1_sb", tag="on1")
                nc.gpsimd.memset(ones1_sb[:], 1.0)

                bq_col = small_sb[:, COL_BQ : COL_BQ + 1]
                bk_col = small_sb[:, COL_BK : COL_BK + 1]
                bv_col = small_sb[:, COL_BV : COL_BV + 1]

                # vaug: one tile, 8 slots of 66 cols (64 data + ones col)
                vaug_all = persist.tile([128, 528], BF16, name="vaug_all", tag="va")
                vaug_r = vaug_all.rearrange("p (i c) -> p i c", i=8, c=66)
                nc.gpsimd.memset(vaug_r[:, :, 64:66], 1.0)

                def vaug_slice(p, mt):
                    i = 4 * p + mt
                    return vaug_all[:, 66 * i : 66 * i + 65]

                # ---------------- conv + LN ----------------
                xt4 = [
                    xt_sb[kt].rearrange(
                        "p (i di j dj) -> p i di j dj", i=32, di=2, j=32, dj=2
                    )
                    for kt in range(4)
                ]
                xz_sb = [
                    persist.tile([128, M], BF16, name=f"xz{ot}", tag=f"xz{ot}")
                    for ot in range(4)
                ]
                xzt_all = persist.tile([128, 8 * 512], BF16, name="xzt_all", tag="xzt")
                xzt_sb = [xzt_all[:, j * 512 : (j + 1) * 512] for j in range(8)]
                xzt_dst = xzt_all.rearrange("p (j c) -> p j c", j=8, c=512)
                s1c = persist.tile([128, 4], F32, name="s1c", tag="s1c")
                s2c = persist.tile([128, 4], F32, name="s2c", tag="s2c")
                mu4 = persist.tile([128, 4], F32, name="mu4", tag="mu4")
                rv4 = persist.tile([128, 4], F32, name="rv4", tag="rv4")
                rs4 = persist.tile([128, 4], F32, name="rs4", tag="rs4")
                dummy = persist.tile([128, 1], F32, name="dummy", tag="dum")

                for ot in range(4):
                    c_ps = pmm.tile([128, 1024], F32, name="c_ps", tag="mm")
                    for h in range(2):
                        first = True
                        for kt in range(4):
                            for di in range(2):
                                for dj in range(2):
                                    tap = 2 * di + dj
                                    nc.tensor.matmul(
                                        c_ps[:, 512 * h : 512 * (h + 1)],
                                        wc_sb[
                                            :,
                                            2048 * ot + 512 * kt + 128 * tap : 2048 * ot
                                            + 512 * kt
                                            + 128 * (tap + 1),
                                        ],
                                        xt4[kt][:, 16 * h : 16 * (h + 1), di, :, dj],
                                        start=first,
                                        stop=(kt == 3 and tap == 3),
                                    )
                                    first = False
                    # conv bias is constant along m, so LayerNorm's mean
                    # subtraction cancels it exactly -- skip it and read the
                    # PSUM tile directly for the LN stats and normalize.
                    nc.vector.tensor_reduce(
                        s1c[:, ot : ot + 1], c_ps[:], axis=AX, op=OP.add
                    )
                    sq_scr = stage.tile([128, M], F32, name="sq_scr", tag="sq", bufs=2)
                    nc.scalar.activation(
                        sq_scr[:], c_ps[:], AF.Square,
                        accum_out=s2c[:, ot : ot + 1],
                    )
                    nc.vector.tensor_scalar_mul(
                        mu4[:, ot : ot + 1], s1c[:, ot : ot + 1], 1.0 / M
                    )
                    mu2 = stage.tile([128, 1], F32, name="mu2", tag="mu2")
                    nc.vector.tensor_mul(
                        mu2[:], mu4[:, ot : ot + 1], mu4[:, ot : ot + 1]
                    )
                    ve = stage.tile([128, 1], F32, name="ve", tag="ve")
                    nc.vector.tensor_scalar(
                        out=ve[:],
                        in0=s2c[:, ot : ot + 1],
                        scalar1=1.0 / M,
                        scalar2=EPS,
                        op0=OP.mult,
                        op1=OP.add,
                    )
                    nc.vector.tensor_sub(ve[:], ve[:], mu2[:])
                    nc.vector.reciprocal(rv4[:, ot : ot + 1], ve[:])
                    # rsqrt(v) = sqrt(1/v) via DVE Newton (3 iters, linear seed);
                    # keeps ACT on the exp table (no sqrt table switch).
                    rs_c = rs4[:, ot : ot + 1]
                    nc.vector.tensor_scalar(
                        out=rs_c, in0=rv4[:, ot : ot + 1],
                        scalar1=0.4163, scalar2=0.519, op0=OP.mult, op1=OP.add,
                    )
                    for _it in range(2):
                        nt = stage.tile([128, 1], F32, name="nt", tag="nt")
                        nc.vector.tensor_mul(nt[:], rs_c, rs_c)
                        nc.vector.tensor_mul(nt[:], nt[:], ve[:])
                        nc.vector.tensor_scalar(
                            out=nt[:], in0=nt[:],
                            scalar1=-0.5, scalar2=1.5, op0=OP.mult, op1=OP.add,
                        )
                        nc.vector.tensor_mul(rs_c, rs_c, nt[:])
                    # normalize (f32 -> bf16) + transpose this ot immediately
                    nc.vector.tensor_scalar(
                        out=xz_sb[ot][:],
                        in0=c_ps[:],
                        scalar1=mu4[:, ot : ot + 1],
                        scalar2=rs_c,
                        op0=OP.subtract,
                        op1=OP.mult,
                    )

                # ---------------- transpose z -> [m, cpos] ----------------
                for ot in range(4):
                    tpa = pmm.tile([128, 512], BF16, name="tpa", tag="mm")
                    tpb = pmm.tile([128, 512], BF16, name="tpb", tag="mm")
                    for j in range(8):
                        dst = tpa if j < 4 else tpb
                        nc.tensor.transpose(
                            dst[:, 128 * (j % 4) : 128 * (j % 4 + 1)],
                            xz_sb[ot][:, 128 * j : 128 * (j + 1)],
                            eye_sb[:],
                        )
                    nc.vector.tensor_copy(
                        xzt_dst[:, 0:4, 128 * ot : 128 * (ot + 1)],
                        tpa.rearrange("p (j c) -> p j c", j=4, c=128),
                    )
                    nc.vector.tensor_copy(
                        xzt_dst[:, 4:8, 128 * ot : 128 * (ot + 1)],
                        tpb.rearrange("p (j c) -> p j c", j=4, c=128),
                    )

                # ---------------- KV ----------------
                kT_sb = persist.tile([128, 512], BF16, name="kT_sb", tag="kT")
                vT_sb = persist.tile([128, 512], BF16, name="vT_sb", tag="vT")
                kv_ps = pmm.tile([128, 1024], F32, name="kv_ps", tag="mm")
                for half, lo in ((0, 0), (1, 128)):
                    for m in range(8):
                        nc.tensor.matmul(
                            kv_ps[:, 512 * half : 512 * (half + 1)],
                            wkv_sb[:, 256 * m + lo : 256 * m + lo + 128],
                            xzt_sb[m][:],
                            start=(m == 0),
                            stop=(m == 7),
                        )
                nc.vector.tensor_scalar_add(kT_sb[:], kv_ps[:, 0:512], bk_col)
                nc.vector.tensor_scalar_add(vT_sb[:], kv_ps[:, 512:1024], bv_col)

                # v -> [cpos, hd] augmented tiles
                tpv8 = pstp.tile([128, 512], BF16, name="tpv8", tag="tp")
                for p in range(2):
                    for mt in range(4):
                        i = 4 * p + mt
                        nc.tensor.transpose(
                            tpv8[:, 64 * i : 64 * (i + 1)],
                            vT_sb[64 * p : 64 * (p + 1), 128 * mt : 128 * (mt + 1)],
                            eye_sb[64 * p : 64 * (p + 1), 64 * p : 64 * (p + 1)],
                        )
                nc.vector.tensor_copy(
                    vaug_r[:, :, 0:64],
                    tpv8.rearrange("p (i c) -> p i c", i=8, c=64),
                )

                # ---------------- attention ----------------
                qt_sb = persist.tile([128, N], BF16, name="qt_sb", tag="qt")
                avT65 = [
                    persist.tile([65, N], BF16, name=f"avT65_{p}", tag=f"av{p}")
                    for p in range(2)
                ]
                tpS = [
                    persist.tile([128, 8 * 264], BF16, name=f"tpS{p}", tag=f"tS{p}")
                    for p in range(2)
                ]
                tpS_r = [t.rearrange("p (i c) -> p i c", i=32, c=66) for t in tpS]
                recip_sb = [
                    persist.tile([128, 32], F32, name=f"recip{p}", tag=f"rc{p}")
                    for p in range(2)
                ]
                out2dT = [
                    [
                        persist.tile(
                            [128, 512], BF16, name=f"o2dT{p}_{ct}", tag=f"o2{p}{ct}"
                        )
                        for ct in range(4)
                    ]
                    for p in range(2)
                ]

                def q_chunk(ch, q_ps):
                    half = q_ps[:, 512 * (ch % 2) : 512 * (ch % 2 + 1)]
                    for k in range(4):
                        nc.tensor.matmul(
                            half,
                            wq_sb[:, 128 * k : 128 * (k + 1)],
                            xt_sb[k][:, 512 * ch : 512 * (ch + 1)],
                            start=(k == 0),
                            stop=(k == 3),
                        )
                    nc.vector.tensor_scalar_add(
                        qt_sb[:, 512 * ch : 512 * (ch + 1)], half, bq_col
                    )

                q_ps = pmm.tile([128, 1024], F32, name="q_ps", tag="mm")
                q_chunk(0, q_ps)
                q_chunk(1, q_ps)

                out_r = out.rearrange("(P d s) c -> P s d c", P=2, d=64, s=8)

                def proj_group(rt):
                    pr_ps = pmm.tile([128, 1024], F32, name="pr_ps", tag="mm")
                    for p in range(2):
                        half = pr_ps[:, 512 * p : 512 * (p + 1)]
                        for ct in range(4):
                            nc.tensor.matmul(
                                half,
                                out2dT[p][ct][:, 128 * rt : 128 * (rt + 1)],
                                wproj_sb[:, 512 * ct : 512 * (ct + 1)],
                                start=(ct == 0),
                                stop=False,
                            )
                        nc.tensor.matmul(
                            half, ones1_sb[:], bproj_sb[:], start=False, stop=True
                        )
                        of = stage.tile([128, 512], F32, name="of", tag="of", bufs=3)
                        if p == 0:
                            nc.scalar.activation(of[:], half, AF.Identity)
                        else:
                            nc.vector.tensor_copy(of[:], half)
                        eng = nc.scalar if p == 0 else nc.sync
                        eng.dma_start(out_r[p, 2 * rt : 2 * rt + 2, :, :], of[:])

                for ch in range(8):
                    if True:
                        pass
                    # Q for chunk ch+2 (pair tiles at even boundaries)
                    if ch % 2 == 0 and ch + 2 < 8:
                        q_ps = pmm.tile([128, 1024], F32, name="q_ps", tag="mm")
                        q_chunk(ch + 2, q_ps)
                        q_chunk(ch + 3, q_ps)

                    for p in range(2):
                        phats = []
                        for mt2 in range(2):
                            s_ps = pmm.tile([128, 1024], F32, name="s_ps", tag="mm")
                            for mti in range(2):
                                mt = 2 * mt2 + mti
                                nc.tensor.matmul(
                                    s_ps[:, 512 * mti : 512 * (mti + 1)],
                                    kT_sb[
                                        64 * p : 64 * (p + 1),
                                        128 * mt : 128 * (mt + 1),
                                    ],
                                    qt_sb[
                                        64 * p : 64 * (p + 1),
                                        512 * ch : 512 * (ch + 1),
                                    ],
                                    start=True,
                                    stop=True,
                                )
                            ph = stage.tile(
                                [128, 1024], BF16, name="phat", tag="ph", bufs=6
                            )
                            nc.scalar.activation(ph[:], s_ps[:], AF.Exp, scale=0.125)
                            phats.append(ph)
                        av_ps = psav.tile([65, 512], F32, name="av_ps", tag="av")
                        for mt in range(4):
                            nc.tensor.matmul(
                                av_ps[:],
                                vaug_slice(p, mt),
                                phats[mt // 2][:, 512 * (mt % 2) : 512 * (mt % 2 + 1)],
                                start=(mt == 0),
                                stop=(mt == 3),
                            )
                        nc.vector.tensor_copy(
                            avT65[p][:, 512 * ch : 512 * (ch + 1)], av_ps[:]
                        )

                    # per-chunk tail: transpose AV^T+den (both heads into one
                    # psum tile so head-1 never stalls on head-0's copy), recip,
                    # scatter
                    tp528 = pstp.tile([128, 528], BF16, name="tp528", tag="tp")
                    for p in range(2):
                        for ct in range(4):
                            i = 4 * ch + ct
                            nc.tensor.transpose(
                                tp528[:, 264 * p + 66 * ct : 264 * p + 66 * ct + 65],
                                avT65[p][0:65, 128 * i : 128 * (i + 1)],
                                eye_sb[0:65, 0:65],
                            )
                    for p in range(2):
                        nc.vector.tensor_copy(
                            tpS[p][:, 264 * ch : 264 * (ch + 1)],
                            tp528[:, 264 * p : 264 * (p + 1)],
                        )
                        nc.vector.reciprocal(
                            recip_sb[p][:, 4 * ch : 4 * ch + 4],
                            tpS_r[p][:, 4 * ch : 4 * ch + 4, 64],
                        )
                        for ct in range(4):
                            i = 4 * ch + ct
                            nc.gpsimd.tensor_scalar_mul(
                                out2dT[p][ct][:, 64 * ch : 64 * (ch + 1)],
                                tpS[p][:, 66 * i : 66 * i + 64],
                                recip_sb[p][:, i : i + 1],
                            )

                    # proj groups 0-2 depend only on chunks 0-5; run them
                    # under chunk 7's ACT/DVE work so only rt=3 is tail.
                    if ch == 6:
                        for rt in range(3):
                            proj_group(rt)

                proj_group(3)



    nc.compile()
    return nc


_NC_CACHE = None


def _get_module():
    global _NC_CACHE
    if _NC_CACHE is None:
        _NC_CACHE = _build_module()
    return _NC_CACHE


def _prep_core_inputs(inputs):
    """Host-side sharding: slice / transpose / cast weights, build 8 in_maps."""
    x = np.asarray(inputs["x"], np.float32)
    Wq = np.asarray(inputs["Wq"], np.float32)
    bq = np.asarray(inputs["bq"], np.float32)
    Wconv = np.asarray(inputs["Wconv"], np.float32)
    bconv = np.asarray(inputs["bconv"], np.float32)
    gamma = np.asarray(inputs["gamma"], np.float32)
    beta = np.asarray(inputs["beta"], np.float32)
    Wkv = np.asarray(inputs["Wkv"], np.float32)
    bkv = np.asarray(inputs["bkv"], np.float32)
    Wproj = np.asarray(inputs["Wproj"], np.float32)
    bproj = np.asarray(inputs["bproj"], np.float32)

    # x^T per batch, bf16
    xt16 = [np.ascontiguousarray(x[b].T).astype(NP_BF16) for b in range(B)]

    # conv weights: wc[p, ot*2048 + kt*512 + tap*128 + o]
    Wt = Wconv.transpose(1, 2, 3, 0).reshape(4, 128, 2, 2, 4, 128)
    wc = np.ascontiguousarray(
        Wt.transpose(1, 4, 0, 2, 3, 5).reshape(128, 8192)
    ).astype(NP_BF16)

    eye128 = np.eye(128, dtype=np.float32).astype(NP_BF16)
    wproj_p = np.ascontiguousarray(
        Wproj.reshape(4, 128, 512).transpose(1, 0, 2).reshape(128, 2048)
    ).astype(NP_BF16)
    bproj16 = np.ascontiguousarray(bproj.reshape(1, C)).astype(NP_BF16)

    Wkv_eff = Wkv * gamma[:, None]
    bias_row = beta @ Wkv + bkv  # [1024]

    in_maps = []
    for core in range(N_CORES):
        b, g = divmod(core, 4)
        ucols = slice(128 * g, 128 * (g + 1))
        vcols = slice(512 + 128 * g, 512 + 128 * (g + 1))
        small = np.zeros((128, SMALL_COLS), np.float32)
        small[:, COL_BQ] = bq[ucols]
        small[:, COL_BCONV : COL_BCONV + 4] = bconv.reshape(4, 128).T
        small[:, COL_BK] = bias_row[ucols]
        small[:, COL_BV] = bias_row[vcols]
        wq_p = np.ascontiguousarray(
            Wq[:, ucols].reshape(4, 128, 128).transpose(1, 0, 2).reshape(128, 512)
        ).astype(NP_BF16)
        wkv2 = np.concatenate([Wkv_eff[:, ucols], Wkv_eff[:, vcols]], axis=1)
        wkv_p = np.ascontiguousarray(
            wkv2.reshape(8, 128, 256).transpose(1, 0, 2).reshape(128, 2048)
        ).astype(NP_BF16)
        in_maps.append(
            {
                "xt16": xt16[b],
                "wc": wc,
                "wq": wq_p,
                "wkv": wkv_p,
                "wproj": wproj_p,
                "small": small,
                "eye128": eye128,
                "bproj16": bproj16,
            }
        )
    return in_maps


def run_spmd(inputs, **kwargs):
    """Run the SPMD kernel; returns (full_output, BassKernelResults)."""
    nc = _get_module()
    in_maps = _prep_core_inputs(inputs)
    res = run_bass_kernel_spmd(nc, in_maps, core_ids=list(range(N_CORES)), **kwargs)
    full = np.empty((B, N, C), np.float32)
    for core in range(N_CORES):
        b, g = divmod(core, 4)
        full[b, 1024 * g : 1024 * (g + 1), :] = res.results[core]["out"]
    return full, res


def kernel(**inputs) -> np.ndarray:
    full, _ = run_spmd(inputs)
    return full
